# revision 15
# baseline (speedup 1.0000x reference)
"""Trainium2 Bass kernel for nn_DescriptorContrastiveLoss (v3, bf16+fp16).

Contract: kernel(**inputs) takes FULL inputs (as produced by
reference.setup_inputs()) and returns the FULL scalar output.

Sharding: data-parallel over (batch, row-half): core c handles batch c//2,
row-half c%2.  Per core:
  - Phase R: separable trilinear resize in bf16 (fp32 PSUM accum),
    batched DRAM bounces between the d/h/w contraction stages; stage-1
    operands are produced first so stage-1 distances/gather overlap the
    rest of the resize.
  - Phase S: s[n,m] = 2<a_n,b_m> - |b_m|^2 via K=4 bf16 matmuls; ScalarE
    downconverts PSUM fp32 -> SBUF fp16, DVE max/max_index run on fp16
    at 2x rate.  Gathers are issued mid-loop to overlap compute.
  - Phase G: indirect row-gathers of matched target descriptors, cosine
    similarities, local sums.
Host combines the 8 partial sums into the final scalar loss.  The gather
row order is a fixed permutation absorbed into the host-side layout of
the source descriptors (the final reduction is permutation-invariant).
"""
import sys

sys.path.insert(0, '/opt/trn_rl_repo')

import numpy as np
from contextlib import ExitStack

import concourse.bass as bass
import concourse.tile as tile
import concourse.bacc as bacc
import concourse.mybir as mybir
from concourse._compat import with_exitstack
from concourse.bass_utils import run_bass_kernel_spmd

F32 = mybir.dt.float32
F16 = mybir.dt.float16
BF16 = mybir.dt.bfloat16
U32 = mybir.dt.uint32
I16 = mybir.dt.int16
ALU = mybir.AluOpType
ACTF = mybir.ActivationFunctionType

B = 4
C = 3
D = 64          # input volume side
S0, S1 = 16, 8  # stage output sides
N0, N1 = S0 ** 3, S1 ** 3   # 4096, 512
CD = 32         # descriptor channels
NCORES = 8
NI = N0 // 2 + N1 // 2      # 2304 gathered rows per core

# d-slice of the source volume needed per half (with filter support halo)
_SRC_D0 = {0: 0, 1: 28}
_SRC_DN = 36


def _resize_weights(in_size: int, out_size: int) -> np.ndarray:
    """fp32-faithful replica of jax.image resize weights (triangle kernel,
    antialias=True, translation=0).  Returns [in_size, out_size]."""
    scale = out_size / in_size
    inv_scale = np.float32(1.0 / scale)
    kernel_scale = np.float32(max(1.0 / scale, 1.0))
    sample_f = ((np.arange(out_size, dtype=np.float32) + np.float32(0.5))
                * inv_scale - np.float32(0.5))
    x = np.abs(sample_f[None, :]
               - np.arange(in_size, dtype=np.float32)[:, None]) / kernel_scale
    w = np.maximum(np.float32(0), np.float32(1) - x).astype(np.float32)
    tot = w.sum(axis=0, keepdims=True, dtype=np.float32)
    w = np.where(np.abs(tot) > 1000.0 * float(np.finfo(np.float32).eps),
                 w / np.where(tot != 0, tot, 1), 0).astype(np.float32)
    valid = (sample_f >= -0.5) & (sample_f <= in_size - 0.5)
    return np.where(valid[None, :], w, 0).astype(np.float32)


@with_exitstack
def _kern(ctx: ExitStack, tc: tile.TileContext, io: dict):
    nc = tc.nc
    dbg = io.get('_dbg', 0)

    consts = ctx.enter_context(tc.tile_pool(name="consts", bufs=1))
    ident_sb = consts.tile([128, 128], BF16)
    nc.sync.dma_start(ident_sb[:], io['ident'])

    # operand tiles for phase S + index/gather state (live whole kernel)
    args = ctx.enter_context(tc.tile_pool(name="args", bufs=1))
    baug = args.tile([4, N0], BF16)
    aaug = args.tile([4, N0 // 2], BF16)
    b1aug = args.tile([4, N1], BF16)
    a1aug = args.tile([4, N1 // 2], BF16)
    nc.vector.memset(aaug[0:1, :], 1.0)
    nc.vector.memset(a1aug[0:1, :], 1.0)

    idxp = ctx.enter_context(tc.tile_pool(name="idx", bufs=1))
    zi = idxp.tile([128, 18], U32)
    ma_all = idxp.tile([128, 16], F16)
    mb_all = idxp.tile([128, 16], F16)
    ia_all = idxp.tile([128, 16], U32)
    ib_all = idxp.tile([128, 16], U32)
    cond = idxp.tile([128, 16], U32)

    scp = ctx.enter_context(tc.tile_pool(name="scp", bufs=2))
    mta = idxp.tile([128, 8], F16)
    mtb = idxp.tile([128, 8], F16)
    nc.vector.memset(mta[:, 1:8], -65504.0)
    nc.vector.memset(mtb[:, 1:8], -65504.0)

    gp = ctx.enter_context(tc.tile_pool(name="gath", bufs=1))
    sd_sb = gp.tile([128, 18, CD], F32)
    nc.sync.dma_start(sd_sb[:],
                      io['sdc'].rearrange("(t p) c -> p t c", p=128))
    gd = gp.tile([128, 16, 2 * CD], F32)
    gd1 = gp.tile([128, 2, 2 * CD], F32)
    table = gp.tile([128, 128], I16)
    table1 = gp.tile([128, 16], I16)

    def build_table0_half(z16h, half):
        # tableX cols [64*half, 64*half+64]: block j <- z16h[16j:16j+16, 0:8]
        for j in range(8):
            nc.sync.dma_start(table[0:16, 64 * half + 8 * j:
                                    64 * half + 8 * (j + 1)],
                              z16h[16 * j:16 * (j + 1), :])
        for g in range(1, 8):
            nc.sync.dma_start(table[16 * g:16 * (g + 1),
                                    64 * half:64 * (half + 1)],
                              table[0:16, 64 * half:64 * (half + 1)])

    # ---------------- Phase R ----------------
    with tc.tile_pool(name="rw", bufs=1) as rw, \
         tc.tile_pool(name="rvol", bufs=1) as rvol, \
         tc.tile_pool(name="rs1", bufs=1) as rs1, \
         tc.tile_pool(name="rt2", bufs=1) as rt2, \
         tc.tile_pool(name="rs2", bufs=1) as rs2:

        def wtile(name, p, f):
            t = rw.tile([p, f], BF16, name=name + "_sb")
            nc.sync.dma_start(t[:], io[name])
            return t
        wdt_sb = wtile('wdt', 64, 24)
        wds_sb = wtile('wds', _SRC_DN, 12)
        wh0_sb = wtile('wh0', 64, S0)
        wh1_sb = wtile('wh1', 64, S1)
        ww0_sb = wtile('ww0', 64, S0)
        wa0_sb = wtile('wa0', 64, S0)
        ww1_sb = wtile('ww1', 64, S1)
        wa1_sb = wtile('wa1', 64, S1)

        ct_sb = rvol.tile([64, C * D * D], BF16)
        for q in range(4):
            nc.sync.dma_start(ct_sb[:, 3072 * q:3072 * (q + 1)],
                              io['ctb'][:, 3072 * q:3072 * (q + 1)])
        cs_sb = rvol.tile([_SRC_DN, C * D * D], BF16)
        for q in range(3):
            nc.sync.dma_start(cs_sb[:, 4096 * q:4096 * (q + 1)],
                              io['csb'][:, 4096 * q:4096 * (q + 1)])

        s1t = rs1.tile([24, C * D * D], BF16)
        s1s = rs1.tile([12, C * D * D], BF16)
        t2t = rt2.tile([64, C * 24 * D], BF16)
        t2s = rt2.tile([64, C * 12 * D], BF16)

        # L1 (contract d) + per-c bounce store / transposed reload
        with tc.tile_pool(name="psl1", bufs=2, space="PSUM") as psl1, \
             tc.tile_pool(name="psl2", bufs=2, space="PSUM") as psl2:
            for k in range(24):
                sl = slice(512 * k, 512 * (k + 1))
                p1t = psl1.tile([24, 512], F32, tag="p1t")
                nc.tensor.matmul(p1t[:], wdt_sb[:], ct_sb[:, sl],
                                 start=True, stop=True)
                nc.vector.tensor_copy(s1t[:, sl], p1t[:])
                p1s = psl1.tile([12, 512], F32, tag="p1s")
                nc.tensor.matmul(p1s[:], wds_sb[:], cs_sb[:, sl],
                                 start=True, stop=True)
                nc.vector.tensor_copy(s1s[:, sl], p1s[:])
                if k % 8 == 7:
                    c = k // 8
                    nc.sync.dma_start(
                        io['y1t'][c].rearrange("do h w -> do (h w)"),
                        s1t[:, 4096 * c:4096 * (c + 1)])
                    nc.sync.dma_start(
                        io['y1s'][c].rearrange("do h w -> do (h w)"),
                        s1s[:, 4096 * c:4096 * (c + 1)])
                    nc.scalar.dma_start(
                        t2t[:, 1536 * c:1536 * (c + 1)].rearrange(
                            "h (do w) -> h do w", do=24),
                        io['y1t'][c].rearrange("do h w -> h do w"))
                    nc.scalar.dma_start(
                        t2s[:, 768 * c:768 * (c + 1)].rearrange(
                            "h (do w) -> h do w", do=12),
                        io['y1s'][c].rearrange("do h w -> h do w"))

            # L2 (contract h); stage-1 volumes first within each c
            s2t0 = rs2.tile([S0, C * S0 * D], BF16)   # [16, 3072]
            s2t1 = rs2.tile([S1, C * S1 * D], BF16)   # [8, 1536]
            s2s0 = rs2.tile([S0, C * S1 * D], BF16)   # [16, 1536]
            s2s1 = rs2.tile([S1, C * 4 * D], BF16)    # [8, 768]
            for c in range(C):
                p2b = psl2.tile([S1, 512], F32, tag="p2")
                nc.tensor.matmul(p2b[:], wh1_sb[:],
                                 t2t[:, 1536 * c + 1024:1536 * (c + 1)],
                                 start=True, stop=True)
                nc.vector.tensor_copy(s2t1[:, 512 * c:512 * (c + 1)], p2b[:])
                p2d = psl2.tile([S1, 256], F32, tag="p2")
                nc.tensor.matmul(p2d[:], wh1_sb[:],
                                 t2s[:, 768 * c + 512:768 * (c + 1)],
                                 start=True, stop=True)
                nc.vector.tensor_copy(s2s1[:, 256 * c:256 * (c + 1)], p2d[:])
                nc.sync.dma_start(
                    io['y2t1'][c].rearrange("do ho w -> ho do w"),
                    s2t1[:, 512 * c:512 * (c + 1)].rearrange(
                        "ho (do w) -> ho do w", do=S1))
                nc.sync.dma_start(
                    io['y2s1'][c].rearrange("do ho w -> ho do w"),
                    s2s1[:, 256 * c:256 * (c + 1)].rearrange(
                        "ho (do w) -> ho do w", do=4))
                for kk in range(2):
                    p2 = psl2.tile([S0, 512], F32, tag="p2")
                    nc.tensor.matmul(p2[:], wh0_sb[:],
                                     t2t[:, 1536 * c + 512 * kk:
                                         1536 * c + 512 * (kk + 1)],
                                     start=True, stop=True)
                    nc.vector.tensor_copy(
                        s2t0[:, 1024 * c + 512 * kk:
                             1024 * c + 512 * (kk + 1)], p2[:])
                p2c = psl2.tile([S0, 512], F32, tag="p2")
                nc.tensor.matmul(p2c[:], wh0_sb[:],
                                 t2s[:, 768 * c:768 * c + 512],
                                 start=True, stop=True)
                nc.vector.tensor_copy(s2s0[:, 512 * c:512 * (c + 1)], p2c[:])
                nc.sync.dma_start(
                    io['y2t0'][c].rearrange("do ho w -> ho do w"),
                    s2t0[:, 1024 * c:1024 * (c + 1)].rearrange(
                        "ho (do w) -> ho do w", do=S0))
                nc.sync.dma_start(
                    io['y2s0'][c].rearrange("do ho w -> ho do w"),
                    s2s0[:, 512 * c:512 * (c + 1)].rearrange(
                        "ho (do w) -> ho do w", do=S1))

        # L3 (contract w) + stage-1 distances overlap
        with tc.tile_pool(name="l3in", bufs=2) as l3p, \
             tc.tile_pool(name="l3tr", bufs=2) as l3t, \
             tc.tile_pool(name="l3s3", bufs=4) as s3p, \
             tc.tile_pool(name="sq", bufs=1) as sqp, \
             tc.tile_pool(name="sc1", bufs=2) as sc1, \
             tc.tile_pool(name="pstr", bufs=2, space="PSUM") as pstr, \
             tc.tile_pool(name="psl3", bufs=2, space="PSUM") as psl3:

            sqacc = [sqp.tile([128, S0], F32, name="sqacc0"),
                     sqp.tile([128, S0], F32, name="sqacc1")]
            sqtmp = sqp.tile([128, S0], F32)
            sq1 = sqp.tile([64, S1], F32)
            nbf = sqp.tile([128, S0], BF16)

            def l3_chunk(src_rows, n, w_sb, wout, tag):
                t_in = l3p.tile([128, 64], BF16, tag="l3in")
                nc.scalar.dma_start(t_in[0:n, :], src_rows)
                ptr = pstr.tile([64, 128], BF16, tag="ptr")
                nc.tensor.transpose(ptr[:, 0:n], t_in[0:n, :],
                                    ident_sb[0:n, 0:n])
                tr = l3t.tile([64, 128], BF16, tag="l3tr")
                nc.scalar.copy(tr[:, 0:n], ptr[:, 0:n])
                p3 = psl3.tile([128, S0], F32, tag="p3")
                nc.tensor.matmul(p3[0:n, 0:wout], tr[:, 0:n], w_sb[:],
                                 start=True, stop=True)
                s3 = s3p.tile([128, wout], BF16, tag=tag)
                nc.scalar.copy(s3[0:n, :], p3[0:n, 0:wout])
                return s3

            # --- stage-1 operands first ---
            y2t1r = io['y2t1'].rearrange("c do ho w -> (c do ho) w")
            s3a = l3_chunk(y2t1r[0:128], 128, ww1_sb, S1, "s3t1a")
            s3b = l3_chunk(y2t1r[128:192], 64, ww1_sb, S1, "s3t1b")
            nc.sync.dma_start(
                b1aug[1:2, :].rearrange("one (p w) -> one p w", p=64),
                s3a[0:64, 0:S1])
            nc.sync.dma_start(
                b1aug[2:3, :].rearrange("one (p w) -> one p w", p=64),
                s3a[64:128, 0:S1])
            nc.sync.dma_start(
                b1aug[3:4, :].rearrange("one (p w) -> one p w", p=64),
                s3b[0:64, 0:S1])
            nc.vector.tensor_mul(sq1[:], s3a[0:64, 0:S1], s3a[0:64, 0:S1])
            nc.vector.tensor_mul(sqtmp[0:64, 0:S1], s3a[64:128, 0:S1],
                                 s3a[64:128, 0:S1])
            nc.vector.tensor_add(sq1[:], sq1[:], sqtmp[0:64, 0:S1])
            nc.vector.tensor_mul(sqtmp[0:64, 0:S1], s3b[0:64, 0:S1],
                                 s3b[0:64, 0:S1])
            nc.vector.tensor_add(sq1[:], sq1[:], sqtmp[0:64, 0:S1])
            nc.vector.tensor_scalar_mul(sq1[:], sq1[:], -1.0)
            nc.vector.tensor_copy(nbf[0:64, 0:S1], sq1[:])
            nc.sync.dma_start(
                b1aug[0:1, :].rearrange("one (p w) -> one p w", p=64),
                nbf[0:64, 0:S1])

            y2s1r = io['y2s1'].rearrange("c do ho w -> (c do ho) w")
            s3c = l3_chunk(y2s1r[0:96], 96, wa1_sb, S1, "s3s1")
            for c in range(3):
                nc.sync.dma_start(
                    a1aug[1 + c:2 + c, :].rearrange("one (p w) -> one p w",
                                                    p=32),
                    s3c[32 * c:32 * (c + 1), 0:S1])

            # --- stage-1 distances (overlaps t0/s0 L3 below) ---
            with tc.tile_pool(name="ps1", bufs=2, space="PSUM") as ps1p:
                for T in range(2):
                    p1 = ps1p.tile([128, 512], F32, tag="s1")
                    nc.tensor.matmul(p1[:], a1aug[:, 128 * T:128 * (T + 1)],
                                     b1aug[:], start=True, stop=True)
                    s116 = sc1.tile([128, 512], F16, tag="s116")
                    nc.scalar.copy(s116[:], p1[:])
                    m81 = sc1.tile([128, 8], F16, tag="m81")
                    nc.vector.max(m81[:], s116[:])
                    i81 = sc1.tile([128, 8], U32, tag="i81")
                    nc.vector.max_index(i81[:], m81[:], s116[:])
                    nc.scalar.copy(zi[:, 16 + T:17 + T], i81[:, 0:1])
            z1o = gp.tile([128, 2], U32)
            nc.vector.tensor_scalar_add(z1o[:], zi[:, 16:18], N0)
            z116 = gp.tile([128, 2], I16)
            nc.vector.tensor_copy(z116[:], z1o[:])
            for j in range(8):
                nc.sync.dma_start(table1[0:16, 2 * j:2 * (j + 1)],
                                  z116[16 * j:16 * (j + 1), :])
            for g in range(1, 8):
                nc.sync.dma_start(table1[16 * g:16 * (g + 1), :],
                                  table1[0:16, :])
            nc.gpsimd.dma_gather(
                out_ap=gd1[:], in_ap=io['td'], idxs_ap=table1[:],
                num_idxs=N1 // 2, num_idxs_reg=N1 // 2, elem_size=2 * CD,
                single_packet=False)

            # --- stage-0 operands ---
            y2t0r = io['y2t0'].rearrange("c do ho w -> (c do ho) w")

            def t0_chunk(j):
                c, half = j // 2, j % 2
                s3 = l3_chunk(y2t0r[128 * j:128 * (j + 1)], 128, ww0_sb, S0,
                              "s3t0")
                nc.sync.dma_start(
                    baug[1 + c:2 + c,
                         2048 * half:2048 * (half + 1)].rearrange(
                        "one (p w) -> one p w", p=128), s3[:])
                if c == 0:
                    nc.vector.tensor_mul(sqacc[half][:], s3[:], s3[:])
                else:
                    nc.vector.tensor_mul(sqtmp[:], s3[:], s3[:])
                    nc.vector.tensor_add(sqacc[half][:], sqacc[half][:],
                                         sqtmp[:])
                if c == 2:
                    nc.vector.tensor_scalar_mul(sqacc[half][:],
                                                sqacc[half][:], -1.0)
                    nc.vector.tensor_copy(nbf[:], sqacc[half][:])
                    nc.sync.dma_start(
                        baug[0:1, 2048 * half:2048 * (half + 1)].rearrange(
                            "one (p w) -> one p w", p=128), nbf[:])

            for j in (0, 2, 4):     # baug half A
                t0_chunk(j)
            y2s0r = io['y2s0'].rearrange("c do ho w -> (c do ho) w")
            for j in range(3):      # aaug
                s3 = l3_chunk(y2s0r[128 * j:128 * (j + 1)], 128, wa0_sb, S0,
                              "s3s0")
                nc.sync.dma_start(
                    aaug[1 + j:2 + j, :].rearrange("one (p w) -> one p w",
                                                   p=128), s3[:])

            # --- S half A, interleaved with remaining t0 (baug half B) ---
            def s0_half(T, pool, ptag, colbase, stag, mt, m_all, i_all,
                        mtag, itag):
                lhs = aaug[:, 128 * T:128 * (T + 1)]
                p = pool.tile([128, 2048], F32, tag=ptag)
                for j in range(4):
                    nc.tensor.matmul(p[:, 512 * j:512 * (j + 1)], lhs,
                                     baug[:, colbase + 512 * j:
                                          colbase + 512 * (j + 1)],
                                     start=True, stop=True)
                s16 = scp.tile([128, 2048], F16, tag=stag)
                nc.scalar.copy(s16[:], p[:])
                m1 = scp.tile([128, 1], F16, tag=mtag)
                nc.vector.reduce_max(m1[:], s16[:],
                                     axis=mybir.AxisListType.X)
                nc.scalar.copy(mt[:, 0:1], m1[:])
                i8 = scp.tile([128, 8], U32, tag=itag)
                nc.vector.max_index(i8[:], mt[:], s16[:])
                nc.scalar.copy(m_all[:, T:T + 1], m1[:])
                nc.scalar.copy(i_all[:, T:T + 1], i8[:, 0:1])

            with tc.tile_pool(name="psA", bufs=1, space="PSUM") as psa:
                for T in range(16):
                    s0_half(T, psa, "pa", 0, "sa16", mta, ma_all, ia_all,
                            "m1a", "i8a")
                    if T in (2, 5, 8):
                        t0_chunk({2: 1, 5: 3, 8: 5}[T])

    if dbg == 1:
        with tc.tile_pool(name="dbgp", bufs=1) as dp:
            big = dp.tile([16, N0], F32)
            nc.vector.tensor_copy(big[0:4, :], baug[:])
            nc.vector.tensor_copy(big[4:8, 0:2048], aaug[:])
            nc.vector.tensor_copy(big[8:12, 0:512], b1aug[:])
            nc.vector.tensor_copy(big[12:16, 0:256], a1aug[:])
            nc.sync.dma_start(io['sdump'][0:16, :], big[:])
        return

    # ---------------- Phase S: stage-0 distances + argmax ----------------
    def combine_half(lo, hi):
        # z[:, lo:hi] = (mb > ma) ? ib + 2048 : ia ; then table + gather
        nc.vector.tensor_tensor(cond[:, lo:hi], mb_all[:, lo:hi],
                                ma_all[:, lo:hi], ALU.is_gt)
        nc.vector.tensor_scalar_add(ib_all[:, lo:hi], ib_all[:, lo:hi],
                                    2048)
        nc.vector.select(zi[:, lo:hi], cond[:, lo:hi], ib_all[:, lo:hi],
                         ia_all[:, lo:hi])

    with tc.tile_pool(name="psB", bufs=2, space="PSUM") as psb:
        z16h = [gp.tile([128, 8], I16, name="z16ha"),
                gp.tile([128, 8], I16, name="z16hb")]
        for T in range(16):
            s0_half(T, psb, "pb", 2048, "sb16", mtb, mb_all, ib_all,
                    "m1b", "i8b")
            if T == 7 or T == 15:
                half = T // 8
                lo = 8 * half
                combine_half(lo, lo + 8)
                nc.vector.tensor_copy(z16h[half][:], zi[:, lo:lo + 8])
                build_table0_half(z16h[half], half)
                nc.gpsimd.dma_gather(
                    out_ap=gd[:, 8 * half:8 * (half + 1), :],
                    in_ap=io['td'],
                    idxs_ap=table[:, 64 * half:64 * (half + 1)],
                    num_idxs=1024, num_idxs_reg=1024, elem_size=2 * CD,
                    single_packet=False)

    if dbg == 2:
        nc.sync.dma_start(io['zdump'], zi[:])
        return

    # ---------------- Phase G: cosine + local sums ----------------
    if dbg == 31:
        nc.sync.dma_start(io['sdump'][:, 0:16 * 64],
                          gd[:].rearrange("p t c -> p (t c)"))
        nc.sync.dma_start(io['sdump'][:, 1024:1024 + 128],
                          gd1[:].rearrange("p t c -> p (t c)"))
        return

    with tc.tile_pool(name="cosw", bufs=2) as cw, \
         tc.tile_pool(name="psF", bufs=1, space="PSUM") as psf:
        cs01 = gp.tile([128, 2], F32)
        ones_sb = gp.tile([128, 1], F32)
        nc.vector.memset(ones_sb[:], 1.0)
        for st, (gt, lo, nt) in enumerate([(gd, 0, 16), (gd1, 16, 2)]):
            gdv = gt[:, :, 0:CD]
            sdv = sd_sb[:, lo:lo + nt, :]
            prod = cw.tile([128, nt, CD], F32, tag="prod" + str(st))
            num = cw.tile([128, nt], F32, tag="num" + str(st))
            nc.vector.tensor_mul(prod[:], sdv, gdv)
            nc.vector.reduce_sum(num[:], prod[:], axis=mybir.AxisListType.X)
            nc.vector.tensor_mul(prod[:], sdv, sdv)
            sn = cw.tile([128, nt], F32, tag="sn" + str(st))
            nc.vector.reduce_sum(sn[:], prod[:], axis=mybir.AxisListType.X)
            nc.vector.tensor_mul(prod[:], gdv, gdv)
            gn = cw.tile([128, nt], F32, tag="gn" + str(st))
            nc.vector.reduce_sum(gn[:], prod[:], axis=mybir.AxisListType.X)
            nc.scalar.activation(sn[:], sn[:], ACTF.Sqrt)
            nc.scalar.activation(gn[:], gn[:], ACTF.Sqrt)
            nc.vector.tensor_scalar_max(sn[:], sn[:], 1e-8)
            nc.vector.tensor_scalar_max(gn[:], gn[:], 1e-8)
            nc.vector.tensor_mul(sn[:], sn[:], gn[:])
            nc.vector.reciprocal(sn[:], sn[:])
            nc.vector.tensor_mul(num[:], num[:], sn[:])
            nc.vector.reduce_sum(cs01[:, st:st + 1], num[:],
                                 axis=mybir.AxisListType.X)
        pf = psf.tile([2, 1], F32)
        nc.tensor.matmul(pf[:], cs01[:], ones_sb[:], start=True, stop=True)
        of = gp.tile([2, 1], F32)
        nc.scalar.copy(of[:], pf[:])
        nc.sync.dma_start(io['out'].rearrange("(a one) -> a one", one=1),
                          of[:])


def _build_program(dbg=0):
    nc = bacc.Bacc("TRN2", target_bir_lowering=False, debug=False,
                   enable_asserts=True, num_devices=NCORES)
    io = {}
    io['_dbg'] = dbg

    def inp(name, shape, dt=BF16):
        io[name] = nc.dram_tensor(name, list(shape), dt,
                                  kind="ExternalInput").ap()

    inp('ctb', (64, C * D * D))
    inp('csb', (_SRC_DN, C * D * D))
    inp('wdt', (64, 24))
    inp('wds', (_SRC_DN, 12))
    inp('wh0', (64, S0))
    inp('wh1', (64, S1))
    inp('ww0', (64, S0))
    inp('wa0', (64, S0))
    inp('ww1', (64, S1))
    inp('wa1', (64, S1))
    inp('ident', (128, 128))
    inp('td', (N0 + N1, 2 * CD), F32)
    inp('sdc', (NI, CD), F32)
    io['out'] = nc.dram_tensor('out', [2], F32, kind="ExternalOutput").ap()
    io['zdump'] = nc.dram_tensor('zdump', [128, 18], U32,
                                 kind="ExternalOutput").ap()
    io['sdump'] = nc.dram_tensor('sdump', [128, 4096], F32,
                                 kind="ExternalOutput").ap()

    def scratch(name, shape):
        kw = {'kind': 'ExternalOutput'} if dbg >= 1 else {}
        io[name] = nc.dram_tensor(name, list(shape), BF16, **kw).ap()

    scratch('y1t', (C, 24, D, D))
    scratch('y1s', (C, 12, D, D))
    scratch('y2t0', (C, S0, S0, D))
    scratch('y2t1', (C, S1, S1, D))
    scratch('y2s0', (C, S1, S0, D))
    scratch('y2s1', (C, 4, S1, D))

    with tile.TileContext(nc, trace_sim=False) as tc:
        _kern(tc, io)
    nc.compile()
    return nc


_CACHE = {}


def _program(dbg=0):
    key = ('nc', dbg)
    if key not in _CACHE:
        _CACHE[key] = _build_program(dbg)
    return _CACHE[key]


def _gather_perm():
    # stage0: half-tables of 8 T-columns each
    i = np.arange(N0 // 2)
    ih = i % 1024
    n0 = (128 * (ih % 128 // 16) + 16 * (ih // 128) + ih % 16
          + 1024 * (i // 1024))
    i1 = np.arange(N1 // 2)
    n1 = 128 * ((i1 % 32) // 16) + 16 * (i1 // 32) + i1 % 16
    return n0, n1


def _host_inputs(canonical_source, canonical_target, src_desc0, tgt_desc0,
                 src_desc1, tgt_desc1):
    import ml_dtypes
    bf = ml_dtypes.bfloat16
    w0 = _resize_weights(D, S0)   # [64,16]
    w1 = _resize_weights(D, S1)   # [64,8]
    wdt = np.concatenate([w0, w1], axis=1).astype(bf)
    ident = np.eye(128, dtype=np.float32).astype(bf)
    n0, n1 = _gather_perm()
    in_maps = []
    for core in range(NCORES):
        b, h = divmod(core, 2)
        d0 = _SRC_D0[h]
        wds = np.concatenate([w0[d0:d0 + _SRC_DN, 8 * h:8 * h + 8],
                              w1[d0:d0 + _SRC_DN, 4 * h:4 * h + 4]],
                             axis=1).astype(bf)
        td = np.concatenate([
            np.pad(tgt_desc0[b].reshape(CD, N0).T, ((0, 0), (0, CD))),
            np.pad(tgt_desc1[b].reshape(CD, N1).T, ((0, 0), (0, CD)))],
            axis=0).astype(np.float32)
        sd0loc = src_desc0[b].reshape(CD, N0).T[h * 2048:(h + 1) * 2048]
        sd1loc = src_desc1[b].reshape(CD, N1).T[h * 256:(h + 1) * 256]
        sdc = np.concatenate([sd0loc[n0], sd1loc[n1]],
                             axis=0).astype(np.float32)
        m = {
            'ctb': np.ascontiguousarray(
                canonical_target[b].transpose(1, 0, 2, 3).reshape(
                    D, C * D * D)).astype(bf),
            'csb': np.ascontiguousarray(
                canonical_source[b][:, d0:d0 + _SRC_DN].transpose(
                    1, 0, 2, 3).reshape(_SRC_DN, C * D * D)).astype(bf),
            'wdt': wdt, 'wds': wds,
            'wh0': w0.astype(bf), 'wh1': w1.astype(bf),
            'ww0': w0.astype(bf), 'wa0': (2.0 * w0).astype(bf),
            'ww1': w1.astype(bf), 'wa1': (2.0 * w1).astype(bf),
            'ident': ident,
            'td': td,
            'sdc': sdc,
        }
        in_maps.append(m)
    return in_maps


def kernel(dbg=0, **inputs):
    inputs = {k: np.asarray(v, dtype=np.float32) for k, v in inputs.items()}
    nc = _program(dbg)
    in_maps = _host_inputs(**inputs)
    kw = {}
    td = globals().get('TRACE_DIR')
    if td:
        import os
        os.makedirs(td, exist_ok=True)
        kw['tmpdir'] = td
    res = run_bass_kernel_spmd(nc, in_maps, list(range(NCORES)), **kw)
    _CACHE['last_res'] = res
    if dbg:
        return None
    parts = np.stack([res.results[c]['out'] for c in range(NCORES)])
    s0 = parts[:, 0].sum(dtype=np.float64)
    s1 = parts[:, 1].sum(dtype=np.float64)
    l0 = np.float32(1.0) - np.float32(s0 / (B * N0))
    l1 = np.float32(1.0) - np.float32(s1 / (B * N1))
    return np.float32((l0 + l1) / 2.0)
